# revision 29
# baseline (speedup 1.0000x reference)
"""Distributed Trainium2 kernel for nn_Attention_2654289789382 (sparse_attention).

Math (reference):
    sigma = sigmoid(x @ W_sigma + b_sigma)           (b, h, n)
    den_i = exp(sigma)+1 ;  r_i = 1/den_i = sigmoid(-sigma)   in (0.2689, 0.5)
    prior[i,j] = softmax_j(-|i-j| * r_i)
    out = (prior @ v) reshaped @ W_out + b_out,  v = x @ W_v

Structure exploited:
  * r_i >= 0.2689  =>  banded attention, band half-width 64: per 128-row
    i-block only 2 j-tiles of 128 (at +-64) contribute.
  * softmax denominator in closed form (two-sided geometric series):
        den_i = 1 + (2z - z^(i+1) - z^(n-i)) / (1-z),  z = exp(-r_i)
  * Q[j,i] = exp(|i-j| * -r_i) built in matmul-rhs layout: r and 1/den are
    staged to DRAM (PE transpose -> flat row write) and broadcast along
    partitions with stride-0 DMA reads, issued just-in-time per (chunk,
    head) so consumers do not wait on later descriptors.  The |dist|
    master is stored negated so ARG = m2r * r needs no extra negation.
  * PE warm-up: memset-sourced dummy matmuls from ~t+3.6us trip the HAM
    clock gate (cold PE runs at 1.2 GHz) with no DMA dependency.
  * Scalar engine runs only Exp activations (sigmoid via exp + DVE
    reciprocal; sigma bias folded in as a host-precomputed exp(-b) factor)
    -> a single ACT table load.
  * Projection accumulates per head pair into per-block PSUM as soon as
    each head pair is normalized, so almost nothing remains after the
    last Exp; b_out is added via a K=1 ones matmul.

Sharding: 8 cores = 4 batches x 2 sequence halves; no collectives.
"""

import numpy as np
import ml_dtypes

import concourse.bass as bass
import concourse.mybir as mybir
import concourse.tile as tile
from concourse import bacc
from concourse.bass_utils import run_bass_kernel_spmd

F32 = mybir.dt.float32
BF16 = mybir.dt.bfloat16

B, N, D = 4, 2048, 512
H, DH = 8, 64
HALF = N // 2            # 1024 rows per core
PAD = 128                # zero-pad rows at each end of the j range
NJROWS = HALF + 2 * PAD  # 1280 padded j rows per core
NBLK = HALF // 128       # 8 i-blocks per core
NVT = 9                  # V tiles at odd 64-offsets (rows 64k..64k+128, k odd)

_nc_cache = None


def _build_nc():
    nc = bacc.Bacc("TRN2", target_bir_lowering=False, debug=False)

    xTs = nc.dram_tensor("xTs", [128, 4 * HALF], BF16, kind="ExternalInput")
    xTp = nc.dram_tensor("xTp", [128, 4 * 128], BF16, kind="ExternalInput")
    Wvb = nc.dram_tensor("Wvb", [128, 4 * D], BF16, kind="ExternalInput")
    Wob = nc.dram_tensor("Wob", [128, 4 * D], BF16, kind="ExternalInput")
    Wsb = nc.dram_tensor("Wsb", [128, 4 * H], BF16, kind="ExternalInput")
    ones1 = nc.dram_tensor("ones1", [1, 128], BF16, kind="ExternalInput")
    expnb = nc.dram_tensor("expnb", [128, NBLK * H], F32, kind="ExternalInput")
    boutr = nc.dram_tensor("boutr", [1, D], BF16, kind="ExternalInput")
    m2r = nc.dram_tensor("m2r", [128, 256], BF16, kind="ExternalInput")  # NEGATIVE |dist|
    ivp1 = nc.dram_tensor("ivp1", [128, NBLK * H], F32, kind="ExternalInput")
    ivnm = nc.dram_tensor("ivnm", [128, NBLK * H], F32, kind="ExternalInput")
    identb = nc.dram_tensor("identb", [128, 128], BF16, kind="ExternalInput")
    out = nc.dram_tensor("out", [HALF, D], BF16, kind="ExternalOutput")
    r_d = nc.dram_tensor("r_d", [H, HALF], BF16)
    inv_d = nc.dram_tensor("inv_d", [H, HALF], BF16)

    EXP = mybir.ActivationFunctionType.Exp
    MUL = mybir.AluOpType.mult
    ADD = mybir.AluOpType.add

    with tile.TileContext(nc) as tc:
        with (
            tc.tile_pool(name="const", bufs=1) as cpool,
            tc.tile_pool(name="vpool", bufs=1) as vpool,
            tc.tile_pool(name="otpool", bufs=1) as otpool,
            tc.tile_pool(name="sg", bufs=1) as sgpool,
            tc.tile_pool(name="bc", bufs=1) as bcpool,
        ):
            # ------------- PE warm-up first: DMA-free via memset ----------
            wm_t = cpool.tile([128, 128], BF16, tag="wm")
            nc.vector.memset(wm_t[:], 1.0)

            # ---------------- loads ----------
            xTb_t = cpool.tile([128, 4 * NJROWS], BF16, tag="xTb")
            xv = xTb_t[:].rearrange("p (dt j) -> p dt j", dt=4)
            nc.sync.dma_start(xv[:, :, PAD:PAD + HALF], xTs[:, :])
            Wvb_t = cpool.tile([128, 4 * D], BF16, tag="Wvb")
            nc.scalar.dma_start(Wvb_t[:], Wvb[:, :])
            Wsb_t = cpool.tile([128, 4 * H], BF16, tag="Wsb")
            nc.gpsimd.dma_start(Wsb_t[:], Wsb[:, :])
            m2r_t = cpool.tile([128, 256], BF16, tag="m2r")
            nc.gpsimd.dma_start(m2r_t[:], m2r[:, :])
            nc.sync.dma_start(
                xv[:, :, PAD - 64:PAD],
                xTp[:].rearrange("p (dt e j) -> p dt e j", dt=4, e=2)[:, :, 0, :],
            )
            nc.sync.dma_start(
                xv[:, :, PAD + HALF:PAD + HALF + 64],
                xTp[:].rearrange("p (dt e j) -> p dt e j", dt=4, e=2)[:, :, 1, :],
            )
            ones_t = cpool.tile([1, 128], BF16, tag="ones")
            nc.gpsimd.dma_start(ones_t[:], ones1[:, :])
            expnb_t = cpool.tile([128, NBLK * H], F32, tag="expnb")
            nc.gpsimd.dma_start(expnb_t[:], expnb[:, :])
            identb_t = cpool.tile([128, 128], BF16, tag="identb")
            nc.gpsimd.dma_start(identb_t[:], identb[:, :])
            ivp1_t = cpool.tile([128, NBLK * H], F32, tag="ivp1")
            nc.gpsimd.dma_start(ivp1_t[:], ivp1[:, :])
            ivnm_t = cpool.tile([128, NBLK * H], F32, tag="ivnm")
            nc.gpsimd.dma_start(ivnm_t[:], ivnm[:, :])
            Wob_t = cpool.tile([128, 4 * D], BF16, tag="Wob")
            boutr_t = cpool.tile([1, D], BF16, tag="boutr")

            def xT(dt, c0, c1):
                return xTb_t[:, dt * NJROWS + c0:dt * NJROWS + c1]

            with tc.tile_pool(name="wm", bufs=1, space="PSUM") as wmp:
                wps = wmp.tile([128, 128], F32, tag="wps")
                for _ in range(16):
                    nc.tensor.matmul(
                        wps[:], lhsT=wm_t[:], rhs=wm_t[:],
                        start=True, stop=True,
                    )

            # ------------- sigma matmuls + r chain ------------------------
            with tc.tile_pool(name="pss", bufs=1, space="PSUM") as pss:
                sps = pss.tile([128, NBLK * H], F32, tag="sps")
                for b in range(NBLK):
                    for dt in range(4):
                        nc.tensor.matmul(
                            sps[:, b * H:(b + 1) * H],
                            lhsT=xT(dt, PAD + b * 128, PAD + (b + 1) * 128),
                            rhs=Wsb_t[:, dt * H:(dt + 1) * H],
                            start=(dt == 0), stop=(dt == 3),
                        )

                # sigma = 1/(1+exp(-s)*exp(-b)); r = 1/(1+exp(sigma))
                e1 = sgpool.tile([128, NBLK * H], F32, tag="e1")
                nc.scalar.activation(e1[:], sps[:], EXP, scale=-1.0)
                t0 = sgpool.tile([128, NBLK * H], F32, tag="t0")
                nc.vector.tensor_mul(t0[:], e1[:], expnb_t[:])
                nc.vector.tensor_scalar(t0[:], t0[:], 1.0, None, ADD)
                sig = sgpool.tile([128, NBLK * H], F32, tag="sig")
                nc.vector.reciprocal(sig[:], t0[:])
                e2 = sgpool.tile([128, NBLK * H], F32, tag="e2")
                nc.scalar.activation(e2[:], sig[:], EXP)
                t1 = sgpool.tile([128, NBLK * H], F32, tag="t1")
                nc.vector.tensor_scalar(t1[:], e2[:], 1.0, None, ADD)
                r_all = sgpool.tile([128, NBLK * H], F32, tag="r_all")
                nc.vector.reciprocal(r_all[:], t1[:])
                r_hb = sgpool.tile([128, NBLK * H], BF16, tag="r_hb")
                nc.vector.tensor_copy(
                    r_hb[:].rearrange("p (h b) -> p h b", b=NBLK),
                    r_all[:].rearrange("p (b h) -> p h b", h=H),
                )

            vtx = tc.tile_pool(name="psv", bufs=2, space="PSUM")
            psv = vtx.__enter__()
            V_all = vpool.tile([128, NVT * D], BF16, tag="V_all")
            vph = {}

            def v_mm(vt_i):
                pv = psv.tile([128, D], F32, tag="pv", name=f"pv{vt_i}")
                vph[vt_i] = pv
                k = 2 * vt_i + 1
                for dt in range(4):
                    nc.tensor.matmul(
                        pv[:],
                        lhsT=xT(dt, 64 * k, 64 * k + 128),
                        rhs=Wvb_t[:, dt * D:(dt + 1) * D],
                        start=(dt == 0), stop=(dt == 3),
                    )

            def v_copy(vt_i, eng="v"):
                dst = V_all[:, vt_i * D:(vt_i + 1) * D]
                src_ = vph.pop(vt_i)[:]
                if eng == "v":
                    nc.vector.tensor_copy(dst, src_)
                else:
                    nc.scalar.copy(dst, src_)

            v_mm(1)
            v_mm(2)

            # ---- stage r -> DRAM; broadcast helpers ----------------------
            R_all = bcpool.tile([128, H * HALF], BF16, tag="R_all")
            Iv_pair = bcpool.tile([128, 4 * HALF], BF16, tag="Iv_pair")

            def r_bcast(ch, h, eng):
                eng.dma_start(
                    R_all[:, h * HALF + ch * 512:h * HALF + (ch + 1) * 512],
                    r_d[h:h + 1, ch * 512:(ch + 1) * 512].to_broadcast((128, 512)),
                )

            def iv_bcast(hp, ch, eng):
                c0, c1 = ch * 512, (ch + 1) * 512
                eng.dma_start(
                    Iv_pair[0:64, hp * HALF + c0:hp * HALF + c1],
                    inv_d[2 * hp:2 * hp + 1, c0:c1].to_broadcast((64, 512)),
                )
                eng.dma_start(
                    Iv_pair[64:128, hp * HALF + c0:hp * HALF + c1],
                    inv_d[2 * hp + 1:2 * hp + 2, c0:c1].to_broadcast((64, 512)),
                )

            with tc.tile_pool(name="pst", bufs=1, space="PSUM") as pst:
                ptn = pst.tile([64, 128], BF16, tag="ptn")
                nc.tensor.transpose(ptn[:], r_hb[:], identb_t[:])
                rT = sgpool.tile([64, 128], BF16, tag="rT")
                nc.vector.tensor_copy(rT[:], ptn[:])
                nc.sync.dma_start(
                    r_d.ap().rearrange("h (b p) -> (h b) p", p=128), rT[:, :]
                )
                r_bcast(0, 0, nc.sync)
                r_bcast(0, 1, nc.gpsimd)

                v_mm(3)
                v_mm(4)

                # ---- peeled ARG/Exp (ch0, hp0): ahead of inv chain on DVE
                ARG00 = sgpool.tile([128, 2048], BF16, tag="ARG00")
                for hh in range(2):
                    R0 = R_all[:, hh * HALF:hh * HALF + 512]
                    nc.vector.tensor_tensor(
                        ARG00[:, hh * 1024:(hh + 1) * 1024]
                        .rearrange("p (b o q) -> p b o q", b=4, o=2),
                        m2r_t[:]
                        .rearrange("p (one o q) -> p one o q", one=1, o=2)
                        .broadcast_to((128, 4, 2, 128)),
                        R0.rearrange("p (b one q) -> p b one q", b=4, one=1)
                        .broadcast_to((128, 4, 2, 128)),
                        op=MUL,
                    )
                Q00 = sgpool.tile([128, 2048], BF16, tag="Q00")
                nc.scalar.activation(Q00[:], ARG00[:], EXP)

                # ---- 1/den chain: u = 1 + z - expA - expB; inv = (1-z)/u --
                z = sgpool.tile([128, NBLK * H], F32, tag="z")
                nc.scalar.activation(z[:], r_all[:], EXP, scale=-1.0)
                argA = sgpool.tile([128, NBLK * H], F32, tag="argA")
                nc.vector.tensor_mul(argA[:], r_all[:], ivp1_t[:])
                expA = sgpool.tile([128, NBLK * H], F32, tag="expA")
                nc.scalar.activation(expA[:], argA[:], EXP)
                argB = sgpool.tile([128, NBLK * H], F32, tag="argB")
                nc.vector.tensor_mul(argB[:], r_all[:], ivnm_t[:])
                expB = sgpool.tile([128, NBLK * H], F32, tag="expB")
                nc.scalar.activation(expB[:], argB[:], EXP)
                nc.scalar.dma_start(Wob_t[:], Wob[:, :])
                nc.scalar.dma_start(boutr_t[:], boutr[:, :])
                t2 = sgpool.tile([128, NBLK * H], F32, tag="t2")
                nc.vector.tensor_sub(t2[:], z[:], expA[:])
                nc.vector.tensor_sub(t2[:], t2[:], expB[:])
                u = sgpool.tile([128, NBLK * H], F32, tag="u")
                nc.vector.tensor_scalar(u[:], t2[:], 1.0, None, ADD)
                ru = sgpool.tile([128, NBLK * H], F32, tag="ru")
                nc.vector.reciprocal(ru[:], u[:])
                w = sgpool.tile([128, NBLK * H], F32, tag="w")
                nc.vector.tensor_scalar(w[:], z[:], -1.0, 1.0, MUL, ADD)
                inv_hb = sgpool.tile([128, NBLK * H], BF16, tag="inv_hb")
                nc.vector.tensor_tensor(
                    inv_hb[:].rearrange("p (h b) -> p h b", b=NBLK),
                    w[:].rearrange("p (b h) -> p h b", h=H),
                    ru[:].rearrange("p (b h) -> p h b", h=H),
                    op=MUL,
                )
                pti = pst.tile([64, 128], BF16, tag="pti")
                nc.tensor.transpose(pti[:], inv_hb[:], identb_t[:])
                iT = sgpool.tile([64, 128], BF16, tag="iT")
                nc.vector.tensor_copy(iT[:], pti[:])
                nc.gpsimd.dma_start(
                    inv_d.ap().rearrange("h (b p) -> (h b) p", p=128), iT[:, :]
                )

            iv_bcast(0, 0, nc.gpsimd)
            r_bcast(0, 2, nc.sync)
            r_bcast(0, 3, nc.gpsimd)

            v_mm(0)
            v_copy(1)
            v_copy(2)
            v_copy(3)
            v_copy(4)
            v_copy(0, "s")

            outT_t = []
            for t in range(4):
                oT = otpool.tile([128, HALF], BF16, tag=f"oT{t}")
                outT_t.append(oT)

            def Vs(t, h):
                return V_all[:, t * D + h * 64:t * D + (h + 1) * 64]

            # ---------------- main loop (2 chunks of 4 i-blocks) ----------
            with (
                tc.tile_pool(name="ap", bufs=3) as apool,
                tc.tile_pool(name="qp", bufs=3) as qpool,
                tc.tile_pool(name="fin", bufs=3) as fpool,
                tc.tile_pool(name="psa", bufs=1, space="PSUM") as psa,
                tc.tile_pool(name="psf", bufs=4, space="PSUM") as psf,
            ):
                pfs = {}

                def arg_exp(ch, hp):
                    ARG = apool.tile(
                        [128, 2048], BF16, tag="ARG", name=f"A{ch}{hp}"
                    )
                    for hh in range(2):
                        h = 2 * hp + hh
                        R = R_all[:, h * HALF + ch * 512:
                                  h * HALF + (ch + 1) * 512]
                        nc.vector.tensor_tensor(
                            ARG[:, hh * 1024:(hh + 1) * 1024]
                            .rearrange("p (b o q) -> p b o q", b=4, o=2),
                            m2r_t[:]
                            .rearrange("p (one o q) -> p one o q", one=1, o=2)
                            .broadcast_to((128, 4, 2, 128)),
                            R.rearrange("p (b one q) -> p b one q", b=4, one=1)
                            .broadcast_to((128, 4, 2, 128)),
                            op=MUL,
                        )
                    Q = qpool.tile([128, 2048], BF16, tag="Q", name=f"Q{ch}{hp}")
                    nc.scalar.activation(Q[:], ARG[:], EXP)
                    return Q

                def av_norm(ch, hp, Q):
                    pav = psa.tile(
                        [128, 512], F32, tag="pav", name=f"pav{ch}{hp}"
                    )
                    for bi in range(4):
                        b = ch * 4 + bi
                        c0 = bi * 128
                        for hh in range(2):
                            h = 2 * hp + hh
                            for o in range(2):
                                nc.tensor.matmul(
                                    pav[hh * 64:(hh + 1) * 64, c0:c0 + 128],
                                    lhsT=Vs(b + o, h),
                                    rhs=Q[:, hh * 1024 + bi * 256 + o * 128:
                                          hh * 1024 + bi * 256 + (o + 1) * 128],
                                    start=(o == 0),
                                    stop=(o == 1),
                                )
                    nc.vector.tensor_tensor(
                        outT_t[hp][:, ch * 512:(ch + 1) * 512],
                        pav[:],
                        Iv_pair[:, hp * HALF + ch * 512:
                                hp * HALF + (ch + 1) * 512],
                        op=MUL,
                    )

                def proj_acc(ch, hp):
                    # accumulate this head pair into each block's projection
                    for bi in range(4):
                        b = ch * 4 + bi
                        cols = slice(b * 128, (b + 1) * 128)
                        if hp == 0:
                            pfs[b] = psf.tile(
                                [128, D], F32, tag="pf", name=f"pf{b}"
                            )
                        nc.tensor.matmul(
                            pfs[b][:],
                            lhsT=outT_t[hp][:, cols],
                            rhs=Wob_t[:, hp * D:(hp + 1) * D],
                            start=(hp == 0),
                            stop=False,
                        )
                        if hp == 3:
                            nc.tensor.matmul(
                                pfs[b][:], lhsT=ones_t[:], rhs=boutr_t[:],
                                start=False, stop=True,
                            )
                            fin = fpool.tile(
                                [128, D], BF16, tag="fin", name=f"fin{b}"
                            )
                            if ch == 0:
                                nc.vector.tensor_copy(fin[:], pfs[b][:])
                            else:
                                nc.scalar.copy(fin[:], pfs[b][:])
                            nc.sync.dma_start(out[cols, :], fin[:])

                av_norm(0, 0, Q00)
                proj_acc(0, 0)

                for hp in range(1, 4):
                    nh = 2 * hp + 2
                    if nh < H:
                        r_bcast(0, nh, nc.sync)
                        r_bcast(0, nh + 1, nc.gpsimd)
                    else:
                        r_bcast(1, 0, nc.sync)
                        r_bcast(1, 1, nc.gpsimd)
                    iv_bcast(hp, 0, nc.gpsimd)
                    Q = arg_exp(0, hp)
                    v_mm(4 + hp)
                    av_norm(0, hp, Q)
                    v_copy(4 + hp)
                    proj_acc(0, hp)
                v_mm(8)
                v_copy(8)

                for hp in range(4):
                    nh = 2 * hp + 2
                    if nh < H:
                        r_bcast(1, nh, nc.sync)
                        r_bcast(1, nh + 1, nc.gpsimd)
                    iv_bcast(hp, 1, nc.gpsimd)
                    Q = arg_exp(1, hp)
                    av_norm(1, hp, Q)
                    proj_acc(1, hp)

            vtx.__exit__(None, None, None)

    nc.compile()
    return nc


def _make_in_maps(x, W_v, W_sigma, b_sigma, W_out, b_out):
    bf = ml_dtypes.bfloat16
    m2r1 = np.empty((128, 256), dtype=np.float32)
    p = np.arange(128, dtype=np.float32)[:, None]
    q = np.arange(128, dtype=np.float32)[None, :]
    for o in range(2):
        m2r1[:, o * 128:(o + 1) * 128] = -np.abs(q - p + 64.0 - 128.0 * o)
    m2r = m2r1.astype(bf)
    identb = np.eye(128, dtype=np.float32).astype(bf)

    def dt_interleave(W):  # [512, F] -> [128, 4*F]
        F = W.shape[1]
        return np.ascontiguousarray(
            W.reshape(4, 128, F).transpose(1, 0, 2).reshape(128, 4 * F)
        )

    Wvb = dt_interleave(W_v.astype(bf))
    Wsb = dt_interleave(W_sigma.astype(bf))
    Wob = dt_interleave(W_out.astype(bf))
    ones1 = np.ones((1, 128), dtype=np.float32).astype(bf)
    # exp(-b_sigma) replicated in (block, head) column layout
    expnb = np.tile(np.exp(-b_sigma)[None, :], (128, NBLK)).astype(np.float32)
    boutr = b_out[None, :].astype(bf)

    in_maps = []
    for c in range(8):
        bb, half = c // 2, c % 2
        i_start = half * HALF
        xp = np.zeros((NJROWS, D), dtype=np.float32)
        j_lo = max(0, i_start - PAD)
        j_hi = min(N, i_start + HALF + PAD)
        xp[j_lo - (i_start - PAD):j_hi - (i_start - PAD)] = x[bb, j_lo:j_hi]
        xpT = np.ascontiguousarray(xp.T).astype(bf)           # [512, NJROWS]
        xTs = dt_interleave(xpT[:, PAD:PAD + HALF])           # [128, 4*1024]
        xTp = dt_interleave(np.concatenate(
            [xpT[:, PAD - 64:PAD], xpT[:, PAD + HALF:PAD + HALF + 64]], axis=1
        ))                                                     # [128, 4*128]

        pcol = np.arange(128, dtype=np.float32)[:, None]
        blk = np.arange(NBLK, dtype=np.float32)[None, :]
        i_abs = i_start + blk * 128 + pcol                     # [128, NBLK]
        ivp1 = np.repeat(-(i_abs + 1.0), H, axis=1).astype(np.float32)
        ivnm = np.repeat(-(float(N) - i_abs), H, axis=1).astype(np.float32)

        in_maps.append(
            {
                "xTs": xTs,
                "xTp": xTp,
                "Wvb": Wvb,
                "Wsb": Wsb,
                "Wob": Wob,
                "ones1": ones1,
                "expnb": expnb,
                "boutr": boutr,
                "m2r": m2r,
                "ivp1": ivp1,
                "ivnm": ivnm,
                "identb": identb,
            }
        )
    return in_maps


def kernel(x, W_v, W_sigma, b_sigma, W_out, b_out):
    global _nc_cache
    x = np.asarray(x, dtype=np.float32)
    W_v = np.asarray(W_v, dtype=np.float32)
    W_sigma = np.asarray(W_sigma, dtype=np.float32)
    b_sigma = np.asarray(b_sigma, dtype=np.float32)
    W_out = np.asarray(W_out, dtype=np.float32)
    b_out = np.asarray(b_out, dtype=np.float32)

    if _nc_cache is None:
        _nc_cache = _build_nc()
    nc = _nc_cache

    in_maps = _make_in_maps(x, W_v, W_sigma, b_sigma, W_out, b_out)
    res = run_bass_kernel_spmd(nc, in_maps, core_ids=list(range(8)))

    out = np.empty((B, N, D), dtype=np.float32)
    for c in range(8):
        bb, half = c // 2, c % 2
        out[bb, half * HALF:(half + 1) * HALF, :] = (
            res.results[c]["out"].astype(np.float32)
        )
    return out


# revision 30
# speedup vs baseline: 1.0693x; 1.0693x over previous
"""Distributed Trainium2 kernel for nn_Attention_2654289789382 (sparse_attention).

Math (reference):
    sigma = sigmoid(x @ W_sigma + b_sigma)           (b, h, n)
    den_i = exp(sigma)+1 ;  r_i = 1/den_i = sigmoid(-sigma)   in (0.2689, 0.5)
    prior[i,j] = softmax_j(-|i-j| * r_i)
    out = (prior @ v) reshaped @ W_out + b_out,  v = x @ W_v

Structure exploited:
  * r_i >= 0.2689  =>  banded attention, band half-width 64: per 128-row
    i-block only 2 j-tiles of 128 (at +-64) contribute.
  * softmax denominator in closed form (two-sided geometric series):
        den_i = 1 + (2z - z^(i+1) - z^(n-i)) / (1-z),  z = exp(-r_i)
  * Q[j,i] = exp(|i-j| * -r_i) built in matmul-rhs layout: r and 1/den are
    staged to DRAM (PE transpose -> flat row write) and broadcast along
    partitions with stride-0 DMA reads, issued just-in-time per (chunk,
    head) so consumers do not wait on later descriptors.  The |dist|
    master is stored negated so ARG = m2r * r needs no extra negation.
  * PE warm-up: memset-sourced dummy matmuls from ~t+3.6us trip the HAM
    clock gate (cold PE runs at 1.2 GHz) with no DMA dependency.
  * Scalar engine runs only Exp activations (sigmoid via exp + DVE
    reciprocal; sigma bias folded in as a host-precomputed exp(-b) factor)
    -> a single ACT table load.
  * Projection accumulates per head pair into per-block PSUM as soon as
    each head pair is normalized, so almost nothing remains after the
    last Exp; b_out is added via a K=1 ones matmul.

Sharding: 8 cores = 4 batches x 2 sequence halves; no collectives.
"""

import numpy as np
import ml_dtypes

import concourse.bass as bass
import concourse.mybir as mybir
import concourse.tile as tile
from concourse import bacc
from concourse.bass_utils import run_bass_kernel_spmd

F32 = mybir.dt.float32
BF16 = mybir.dt.bfloat16

B, N, D = 4, 2048, 512
H, DH = 8, 64
HALF = N // 2            # 1024 rows per core
PAD = 128                # zero-pad rows at each end of the j range
NJROWS = HALF + 2 * PAD  # 1280 padded j rows per core
NBLK = HALF // 128       # 8 i-blocks per core
NVT = 9                  # V tiles at odd 64-offsets (rows 64k..64k+128, k odd)

_nc_cache = None


def _build_nc():
    nc = bacc.Bacc("TRN2", target_bir_lowering=False, debug=False)

    xTs = nc.dram_tensor("xTs", [128, 4 * HALF], BF16, kind="ExternalInput")
    xTp = nc.dram_tensor("xTp", [128, 4 * 128], BF16, kind="ExternalInput")
    Wvb = nc.dram_tensor("Wvb", [128, 4 * D], BF16, kind="ExternalInput")
    Wob = nc.dram_tensor("Wob", [128, 4 * D], BF16, kind="ExternalInput")
    Wsb = nc.dram_tensor("Wsb", [128, 4 * H], BF16, kind="ExternalInput")
    ones1 = nc.dram_tensor("ones1", [1, 128], BF16, kind="ExternalInput")
    expnb = nc.dram_tensor("expnb", [128, NBLK * H], F32, kind="ExternalInput")
    boutr = nc.dram_tensor("boutr", [1, D], BF16, kind="ExternalInput")
    m2r = nc.dram_tensor("m2r", [128, 256], BF16, kind="ExternalInput")  # NEGATIVE |dist|
    ivp1 = nc.dram_tensor("ivp1", [128, NBLK * H], F32, kind="ExternalInput")
    ivnm = nc.dram_tensor("ivnm", [128, NBLK * H], F32, kind="ExternalInput")
    identb = nc.dram_tensor("identb", [128, 128], BF16, kind="ExternalInput")
    out = nc.dram_tensor("out", [HALF, D], BF16, kind="ExternalOutput")
    r_d = nc.dram_tensor("r_d", [H, HALF], BF16)
    inv_d = nc.dram_tensor("inv_d", [H, HALF], BF16)

    EXP = mybir.ActivationFunctionType.Exp
    MUL = mybir.AluOpType.mult
    ADD = mybir.AluOpType.add

    with tile.TileContext(nc) as tc:
        with (
            tc.tile_pool(name="const", bufs=1) as cpool,
            tc.tile_pool(name="vpool", bufs=1) as vpool,
            tc.tile_pool(name="otpool", bufs=1) as otpool,
            tc.tile_pool(name="sg", bufs=1) as sgpool,
            tc.tile_pool(name="bc", bufs=1) as bcpool,
        ):
            # ------------- PE warm-up first: DMA-free via memset ----------
            wm_t = cpool.tile([128, 128], BF16, tag="wm")
            nc.vector.memset(wm_t[:], 1.0)

            # ---------------- loads ----------
            xTb_t = cpool.tile([128, 4 * NJROWS], BF16, tag="xTb")
            xv = xTb_t[:].rearrange("p (dt j) -> p dt j", dt=4)
            nc.sync.dma_start(xv[:, :, PAD:PAD + HALF], xTs[:, :])
            Wvb_t = cpool.tile([128, 4 * D], BF16, tag="Wvb")
            nc.scalar.dma_start(Wvb_t[:], Wvb[:, :])
            Wsb_t = cpool.tile([128, 4 * H], BF16, tag="Wsb")
            nc.gpsimd.dma_start(Wsb_t[:], Wsb[:, :])
            m2r_t = cpool.tile([128, 256], BF16, tag="m2r")
            nc.gpsimd.dma_start(m2r_t[:], m2r[:, :])
            nc.sync.dma_start(
                xv[:, :, PAD - 64:PAD],
                xTp[:].rearrange("p (dt e j) -> p dt e j", dt=4, e=2)[:, :, 0, :],
            )
            nc.sync.dma_start(
                xv[:, :, PAD + HALF:PAD + HALF + 64],
                xTp[:].rearrange("p (dt e j) -> p dt e j", dt=4, e=2)[:, :, 1, :],
            )
            ones_t = cpool.tile([1, 128], BF16, tag="ones")
            nc.gpsimd.dma_start(ones_t[:], ones1[:, :])
            expnb_t = cpool.tile([128, NBLK * H], F32, tag="expnb")
            nc.gpsimd.dma_start(expnb_t[:], expnb[:, :])
            identb_t = cpool.tile([128, 128], BF16, tag="identb")
            nc.gpsimd.dma_start(identb_t[:], identb[:, :])
            ivp1_t = cpool.tile([128, NBLK * H], F32, tag="ivp1")
            nc.gpsimd.dma_start(ivp1_t[:], ivp1[:, :])
            ivnm_t = cpool.tile([128, NBLK * H], F32, tag="ivnm")
            nc.gpsimd.dma_start(ivnm_t[:], ivnm[:, :])
            Wob_t = cpool.tile([128, 4 * D], BF16, tag="Wob")
            boutr_t = cpool.tile([1, D], BF16, tag="boutr")

            def xT(dt, c0, c1):
                return xTb_t[:, dt * NJROWS + c0:dt * NJROWS + c1]

            with tc.tile_pool(name="wm", bufs=1, space="PSUM") as wmp:
                wps = wmp.tile([128, 128], F32, tag="wps")
                for _ in range(16):
                    nc.tensor.matmul(
                        wps[:], lhsT=wm_t[:], rhs=wm_t[:],
                        start=True, stop=True,
                    )

            # ------------- sigma matmuls + r chain ------------------------
            with tc.tile_pool(name="pss", bufs=1, space="PSUM") as pss:
                sps = pss.tile([128, NBLK * H], F32, tag="sps")
                for b in range(NBLK):
                    for dt in range(4):
                        nc.tensor.matmul(
                            sps[:, b * H:(b + 1) * H],
                            lhsT=xT(dt, PAD + b * 128, PAD + (b + 1) * 128),
                            rhs=Wsb_t[:, dt * H:(dt + 1) * H],
                            start=(dt == 0), stop=(dt == 3),
                        )

                # sigma = 1/(1+exp(-s)*exp(-b)); r = 1/(1+exp(sigma))
                e1 = sgpool.tile([128, NBLK * H], F32, tag="e1")
                nc.scalar.activation(e1[:], sps[:], EXP, scale=-1.0)
                t0 = sgpool.tile([128, NBLK * H], F32, tag="t0")
                nc.vector.tensor_mul(t0[:], e1[:], expnb_t[:])
                nc.vector.tensor_scalar(t0[:], t0[:], 1.0, None, ADD)
                sig = sgpool.tile([128, NBLK * H], F32, tag="sig")
                nc.vector.reciprocal(sig[:], t0[:])
                e2 = sgpool.tile([128, NBLK * H], F32, tag="e2")
                nc.scalar.activation(e2[:], sig[:], EXP)
                t1 = sgpool.tile([128, NBLK * H], F32, tag="t1")
                nc.vector.tensor_scalar(t1[:], e2[:], 1.0, None, ADD)
                r_all = sgpool.tile([128, NBLK * H], F32, tag="r_all")
                nc.vector.reciprocal(r_all[:], t1[:])
                r_hb = sgpool.tile([128, NBLK * H], BF16, tag="r_hb")
                nc.vector.tensor_copy(
                    r_hb[:].rearrange("p (h b) -> p h b", b=NBLK),
                    r_all[:].rearrange("p (b h) -> p h b", h=H),
                )

            vtx = tc.tile_pool(name="psv", bufs=2, space="PSUM")
            psv = vtx.__enter__()
            V_all = vpool.tile([128, NVT * D], BF16, tag="V_all")
            vph = {}

            def v_mm(vt_i):
                pv = psv.tile([128, D], F32, tag="pv", name=f"pv{vt_i}")
                vph[vt_i] = pv
                k = 2 * vt_i + 1
                for dt in range(4):
                    nc.tensor.matmul(
                        pv[:],
                        lhsT=xT(dt, 64 * k, 64 * k + 128),
                        rhs=Wvb_t[:, dt * D:(dt + 1) * D],
                        start=(dt == 0), stop=(dt == 3),
                    )

            def v_copy(vt_i, eng="v"):
                dst = V_all[:, vt_i * D:(vt_i + 1) * D]
                src_ = vph.pop(vt_i)[:]
                if eng == "v":
                    nc.vector.tensor_copy(dst, src_)
                else:
                    nc.scalar.copy(dst, src_)

            v_mm(1)
            v_mm(2)

            # ---- stage r -> DRAM; broadcast helpers ----------------------
            R_all = bcpool.tile([128, H * HALF], BF16, tag="R_all")
            Iv_pair = bcpool.tile([128, 4 * HALF], BF16, tag="Iv_pair")

            def r_bcast(ch, h, eng):
                eng.dma_start(
                    R_all[:, h * HALF + ch * 512:h * HALF + (ch + 1) * 512],
                    r_d[h:h + 1, ch * 512:(ch + 1) * 512].to_broadcast((128, 512)),
                )

            def iv_bcast(hp, ch, eng):
                c0, c1 = ch * 512, (ch + 1) * 512
                eng.dma_start(
                    Iv_pair[0:64, hp * HALF + c0:hp * HALF + c1],
                    inv_d[2 * hp:2 * hp + 1, c0:c1].to_broadcast((64, 512)),
                )
                eng.dma_start(
                    Iv_pair[64:128, hp * HALF + c0:hp * HALF + c1],
                    inv_d[2 * hp + 1:2 * hp + 2, c0:c1].to_broadcast((64, 512)),
                )

            with tc.tile_pool(name="pst", bufs=1, space="PSUM") as pst:
                ptn = pst.tile([64, 128], BF16, tag="ptn")
                nc.tensor.transpose(ptn[:], r_hb[:], identb_t[:])
                rT = sgpool.tile([64, 128], BF16, tag="rT")
                nc.vector.tensor_copy(rT[:], ptn[:])
                nc.sync.dma_start(
                    r_d.ap().rearrange("h (b p) -> (h b) p", p=128), rT[:, :]
                )
                r_bcast(0, 0, nc.sync)
                r_bcast(0, 1, nc.gpsimd)

                v_mm(3)
                v_mm(4)

                # ---- peeled ARG/Exp (ch0, hp0): ahead of inv chain on DVE
                # (v_copies for tiles 1-2 interleave so v_mm(3)/v_mm(4) can
                # reuse their PSUM banks without stalling the PE)
                ARG00 = sgpool.tile([128, 2048], BF16, tag="ARG00")
                for hh in range(2):
                    R0 = R_all[:, hh * HALF:hh * HALF + 512]
                    nc.vector.tensor_tensor(
                        ARG00[:, hh * 1024:(hh + 1) * 1024]
                        .rearrange("p (b o q) -> p b o q", b=4, o=2),
                        m2r_t[:]
                        .rearrange("p (one o q) -> p one o q", one=1, o=2)
                        .broadcast_to((128, 4, 2, 128)),
                        R0.rearrange("p (b one q) -> p b one q", b=4, one=1)
                        .broadcast_to((128, 4, 2, 128)),
                        op=MUL,
                    )
                    v_copy(1 + hh)
                Q00 = sgpool.tile([128, 2048], BF16, tag="Q00")
                nc.scalar.activation(Q00[:], ARG00[:], EXP)

                # ---- 1/den chain: u = 1 + z - expA - expB; inv = (1-z)/u --
                z = sgpool.tile([128, NBLK * H], F32, tag="z")
                nc.scalar.activation(z[:], r_all[:], EXP, scale=-1.0)
                argA = sgpool.tile([128, NBLK * H], F32, tag="argA")
                nc.vector.tensor_mul(argA[:], r_all[:], ivp1_t[:])
                expA = sgpool.tile([128, NBLK * H], F32, tag="expA")
                nc.scalar.activation(expA[:], argA[:], EXP)
                argB = sgpool.tile([128, NBLK * H], F32, tag="argB")
                nc.vector.tensor_mul(argB[:], r_all[:], ivnm_t[:])
                expB = sgpool.tile([128, NBLK * H], F32, tag="expB")
                nc.scalar.activation(expB[:], argB[:], EXP)
                nc.scalar.dma_start(Wob_t[:], Wob[:, :])
                nc.scalar.dma_start(boutr_t[:], boutr[:, :])
                t2 = sgpool.tile([128, NBLK * H], F32, tag="t2")
                nc.vector.tensor_sub(t2[:], z[:], expA[:])
                nc.vector.tensor_sub(t2[:], t2[:], expB[:])
                u = sgpool.tile([128, NBLK * H], F32, tag="u")
                nc.vector.tensor_scalar(u[:], t2[:], 1.0, None, ADD)
                ru = sgpool.tile([128, NBLK * H], F32, tag="ru")
                nc.vector.reciprocal(ru[:], u[:])
                w = sgpool.tile([128, NBLK * H], F32, tag="w")
                nc.vector.tensor_scalar(w[:], z[:], -1.0, 1.0, MUL, ADD)
                inv_hb = sgpool.tile([128, NBLK * H], BF16, tag="inv_hb")
                nc.vector.tensor_tensor(
                    inv_hb[:].rearrange("p (h b) -> p h b", b=NBLK),
                    w[:].rearrange("p (b h) -> p h b", h=H),
                    ru[:].rearrange("p (b h) -> p h b", h=H),
                    op=MUL,
                )
                pti = pst.tile([64, 128], BF16, tag="pti")
                nc.tensor.transpose(pti[:], inv_hb[:], identb_t[:])
                iT = sgpool.tile([64, 128], BF16, tag="iT")
                nc.vector.tensor_copy(iT[:], pti[:])
                nc.gpsimd.dma_start(
                    inv_d.ap().rearrange("h (b p) -> (h b) p", p=128), iT[:, :]
                )

            iv_bcast(0, 0, nc.gpsimd)
            r_bcast(0, 2, nc.sync)
            r_bcast(0, 3, nc.gpsimd)

            v_mm(0)
            v_copy(3)
            v_copy(4)
            v_copy(0, "s")

            outT_t = []
            for t in range(4):
                oT = otpool.tile([128, HALF], BF16, tag=f"oT{t}")
                outT_t.append(oT)

            def Vs(t, h):
                return V_all[:, t * D + h * 64:t * D + (h + 1) * 64]

            # ---------------- main loop (2 chunks of 4 i-blocks) ----------
            with (
                tc.tile_pool(name="ap", bufs=3) as apool,
                tc.tile_pool(name="qp", bufs=3) as qpool,
                tc.tile_pool(name="fin", bufs=3) as fpool,
                tc.tile_pool(name="psa", bufs=1, space="PSUM") as psa,
                tc.tile_pool(name="psf", bufs=4, space="PSUM") as psf,
            ):
                pfs = {}

                def arg_exp(ch, hp):
                    ARG = apool.tile(
                        [128, 2048], BF16, tag="ARG", name=f"A{ch}{hp}"
                    )
                    for hh in range(2):
                        h = 2 * hp + hh
                        R = R_all[:, h * HALF + ch * 512:
                                  h * HALF + (ch + 1) * 512]
                        nc.vector.tensor_tensor(
                            ARG[:, hh * 1024:(hh + 1) * 1024]
                            .rearrange("p (b o q) -> p b o q", b=4, o=2),
                            m2r_t[:]
                            .rearrange("p (one o q) -> p one o q", one=1, o=2)
                            .broadcast_to((128, 4, 2, 128)),
                            R.rearrange("p (b one q) -> p b one q", b=4, one=1)
                            .broadcast_to((128, 4, 2, 128)),
                            op=MUL,
                        )
                    Q = qpool.tile([128, 2048], BF16, tag="Q", name=f"Q{ch}{hp}")
                    nc.scalar.activation(Q[:], ARG[:], EXP)
                    return Q

                def av_norm(ch, hp, Q):
                    pav = psa.tile(
                        [128, 512], F32, tag="pav", name=f"pav{ch}{hp}"
                    )
                    for bi in range(4):
                        b = ch * 4 + bi
                        c0 = bi * 128
                        for hh in range(2):
                            h = 2 * hp + hh
                            for o in range(2):
                                nc.tensor.matmul(
                                    pav[hh * 64:(hh + 1) * 64, c0:c0 + 128],
                                    lhsT=Vs(b + o, h),
                                    rhs=Q[:, hh * 1024 + bi * 256 + o * 128:
                                          hh * 1024 + bi * 256 + (o + 1) * 128],
                                    start=(o == 0),
                                    stop=(o == 1),
                                )
                    nc.vector.tensor_tensor(
                        outT_t[hp][:, ch * 512:(ch + 1) * 512],
                        pav[:],
                        Iv_pair[:, hp * HALF + ch * 512:
                                hp * HALF + (ch + 1) * 512],
                        op=MUL,
                    )

                def proj_acc(ch, hp):
                    # accumulate this head pair into each block's projection
                    for bi in range(4):
                        b = ch * 4 + bi
                        cols = slice(b * 128, (b + 1) * 128)
                        if hp == 0:
                            pfs[b] = psf.tile(
                                [128, D], F32, tag="pf", name=f"pf{b}"
                            )
                        nc.tensor.matmul(
                            pfs[b][:],
                            lhsT=outT_t[hp][:, cols],
                            rhs=Wob_t[:, hp * D:(hp + 1) * D],
                            start=(hp == 0),
                            stop=False,
                        )
                        if hp == 3:
                            nc.tensor.matmul(
                                pfs[b][:], lhsT=ones_t[:], rhs=boutr_t[:],
                                start=False, stop=True,
                            )
                            fin = fpool.tile(
                                [128, D], BF16, tag="fin", name=f"fin{b}"
                            )
                            if ch == 0:
                                nc.vector.tensor_copy(fin[:], pfs[b][:])
                            else:
                                nc.scalar.copy(fin[:], pfs[b][:])
                            nc.sync.dma_start(out[cols, :], fin[:])

                av_norm(0, 0, Q00)
                proj_acc(0, 0)

                for hp in range(1, 4):
                    nh = 2 * hp + 2
                    if nh < H:
                        r_bcast(0, nh, nc.sync)
                        r_bcast(0, nh + 1, nc.gpsimd)
                    else:
                        r_bcast(1, 0, nc.sync)
                        r_bcast(1, 1, nc.gpsimd)
                    iv_bcast(hp, 0, nc.gpsimd)
                    Q = arg_exp(0, hp)
                    v_mm(4 + hp)
                    av_norm(0, hp, Q)
                    v_copy(4 + hp)
                    proj_acc(0, hp)
                v_mm(8)
                v_copy(8)

                for hp in range(4):
                    nh = 2 * hp + 2
                    if nh < H:
                        r_bcast(1, nh, nc.sync)
                        r_bcast(1, nh + 1, nc.gpsimd)
                    iv_bcast(hp, 1, nc.gpsimd)
                    Q = arg_exp(1, hp)
                    av_norm(1, hp, Q)
                    proj_acc(1, hp)

            vtx.__exit__(None, None, None)

    nc.compile()
    return nc


def _make_in_maps(x, W_v, W_sigma, b_sigma, W_out, b_out):
    bf = ml_dtypes.bfloat16
    m2r1 = np.empty((128, 256), dtype=np.float32)
    p = np.arange(128, dtype=np.float32)[:, None]
    q = np.arange(128, dtype=np.float32)[None, :]
    for o in range(2):
        m2r1[:, o * 128:(o + 1) * 128] = -np.abs(q - p + 64.0 - 128.0 * o)
    m2r = m2r1.astype(bf)
    identb = np.eye(128, dtype=np.float32).astype(bf)

    def dt_interleave(W):  # [512, F] -> [128, 4*F]
        F = W.shape[1]
        return np.ascontiguousarray(
            W.reshape(4, 128, F).transpose(1, 0, 2).reshape(128, 4 * F)
        )

    Wvb = dt_interleave(W_v.astype(bf))
    Wsb = dt_interleave(W_sigma.astype(bf))
    Wob = dt_interleave(W_out.astype(bf))
    ones1 = np.ones((1, 128), dtype=np.float32).astype(bf)
    # exp(-b_sigma) replicated in (block, head) column layout
    expnb = np.tile(np.exp(-b_sigma)[None, :], (128, NBLK)).astype(np.float32)
    boutr = b_out[None, :].astype(bf)

    in_maps = []
    for c in range(8):
        bb, half = c // 2, c % 2
        i_start = half * HALF
        xp = np.zeros((NJROWS, D), dtype=np.float32)
        j_lo = max(0, i_start - PAD)
        j_hi = min(N, i_start + HALF + PAD)
        xp[j_lo - (i_start - PAD):j_hi - (i_start - PAD)] = x[bb, j_lo:j_hi]
        xpT = np.ascontiguousarray(xp.T).astype(bf)           # [512, NJROWS]
        xTs = dt_interleave(xpT[:, PAD:PAD + HALF])           # [128, 4*1024]
        xTp = dt_interleave(np.concatenate(
            [xpT[:, PAD - 64:PAD], xpT[:, PAD + HALF:PAD + HALF + 64]], axis=1
        ))                                                     # [128, 4*128]

        pcol = np.arange(128, dtype=np.float32)[:, None]
        blk = np.arange(NBLK, dtype=np.float32)[None, :]
        i_abs = i_start + blk * 128 + pcol                     # [128, NBLK]
        ivp1 = np.repeat(-(i_abs + 1.0), H, axis=1).astype(np.float32)
        ivnm = np.repeat(-(float(N) - i_abs), H, axis=1).astype(np.float32)

        in_maps.append(
            {
                "xTs": xTs,
                "xTp": xTp,
                "Wvb": Wvb,
                "Wsb": Wsb,
                "Wob": Wob,
                "ones1": ones1,
                "expnb": expnb,
                "boutr": boutr,
                "m2r": m2r,
                "ivp1": ivp1,
                "ivnm": ivnm,
                "identb": identb,
            }
        )
    return in_maps


def kernel(x, W_v, W_sigma, b_sigma, W_out, b_out):
    global _nc_cache
    x = np.asarray(x, dtype=np.float32)
    W_v = np.asarray(W_v, dtype=np.float32)
    W_sigma = np.asarray(W_sigma, dtype=np.float32)
    b_sigma = np.asarray(b_sigma, dtype=np.float32)
    W_out = np.asarray(W_out, dtype=np.float32)
    b_out = np.asarray(b_out, dtype=np.float32)

    if _nc_cache is None:
        _nc_cache = _build_nc()
    nc = _nc_cache

    in_maps = _make_in_maps(x, W_v, W_sigma, b_sigma, W_out, b_out)
    res = run_bass_kernel_spmd(nc, in_maps, core_ids=list(range(8)))

    out = np.empty((B, N, D), dtype=np.float32)
    for c in range(8):
        bb, half = c // 2, c % 2
        out[bb, half * HALF:(half + 1) * HALF, :] = (
            res.results[c]["out"].astype(np.float32)
        )
    return out


# revision 33
# speedup vs baseline: 1.0996x; 1.0284x over previous
"""Distributed Trainium2 kernel for nn_Attention_2654289789382 (sparse_attention).

Math (reference):
    sigma = sigmoid(x @ W_sigma + b_sigma)           (b, h, n)
    den_i = exp(sigma)+1 ;  r_i = 1/den_i = sigmoid(-sigma)   in (0.2689, 0.5)
    prior[i,j] = softmax_j(-|i-j| * r_i)
    out = (prior @ v) reshaped @ W_out + b_out,  v = x @ W_v

Structure exploited:
  * r_i >= 0.2689  =>  banded attention, band half-width 64: per 128-row
    i-block only 2 j-tiles of 128 (at +-64) contribute.
  * softmax denominator in closed form (two-sided geometric series):
        den_i = 1 + (2z - z^(i+1) - z^(n-i)) / (1-z),  z = exp(-r_i)
  * Q[j,i] = exp(|i-j| * -r_i) built in matmul-rhs layout: r and 1/den are
    staged to DRAM (PE transpose -> flat row write) and broadcast along
    partitions with stride-0 DMA reads, issued just-in-time per (chunk,
    head) so consumers do not wait on later descriptors.  The |dist|
    master is stored negated so ARG = m2r * r needs no extra negation.
  * PE warm-up: memset-sourced dummy matmuls from ~t+3.6us trip the HAM
    clock gate (cold PE runs at 1.2 GHz) with no DMA dependency.
  * Scalar engine runs only Exp activations (sigmoid via exp + DVE
    reciprocal; sigma bias folded in as a host-precomputed exp(-b) factor)
    -> a single ACT table load.
  * Projection accumulates per head pair into per-block PSUM as soon as
    each head pair is normalized, so almost nothing remains after the
    last Exp; b_out is added via a K=1 ones matmul.

Sharding: 8 cores = 4 batches x 2 sequence halves; no collectives.
"""

import numpy as np
import ml_dtypes

import concourse.bass as bass
import concourse.mybir as mybir
import concourse.tile as tile
from concourse import bacc
from concourse.bass_utils import run_bass_kernel_spmd

F32 = mybir.dt.float32
BF16 = mybir.dt.bfloat16

B, N, D = 4, 2048, 512
H, DH = 8, 64
HALF = N // 2            # 1024 rows per core
PAD = 128                # zero-pad rows at each end of the j range
NJROWS = HALF + 2 * PAD  # 1280 padded j rows per core
NBLK = HALF // 128       # 8 i-blocks per core
NVT = 9                  # V tiles at odd 64-offsets (rows 64k..64k+128, k odd)

_nc_cache = None


def _build_nc():
    nc = bacc.Bacc("TRN2", target_bir_lowering=False, debug=False)

    xTs = nc.dram_tensor("xTs", [128, 4 * HALF], BF16, kind="ExternalInput")
    xTp = nc.dram_tensor("xTp", [128, 4 * 128], BF16, kind="ExternalInput")
    Wvb = nc.dram_tensor("Wvb", [128, 4 * D], BF16, kind="ExternalInput")
    Wob = nc.dram_tensor("Wob", [128, 4 * D], BF16, kind="ExternalInput")
    Wsb = nc.dram_tensor("Wsb", [128, 4 * H], BF16, kind="ExternalInput")
    ones1 = nc.dram_tensor("ones1", [1, 128], BF16, kind="ExternalInput")
    expnb = nc.dram_tensor("expnb", [128, NBLK * H], F32, kind="ExternalInput")
    boutr = nc.dram_tensor("boutr", [1, D], BF16, kind="ExternalInput")
    m2r = nc.dram_tensor("m2r", [128, 256], BF16, kind="ExternalInput")  # NEGATIVE |dist|
    ivp1 = nc.dram_tensor("ivp1", [128, NBLK * H], F32, kind="ExternalInput")
    ivnm = nc.dram_tensor("ivnm", [128, NBLK * H], F32, kind="ExternalInput")
    identb = nc.dram_tensor("identb", [128, 128], BF16, kind="ExternalInput")
    selb = nc.dram_tensor("selb", [16, 16 * 128], BF16, kind="ExternalInput")
    out = nc.dram_tensor("out", [HALF, D], BF16, kind="ExternalOutput")
    r_d = nc.dram_tensor("r_d", [H, HALF], BF16)
    inv_d = nc.dram_tensor("inv_d", [H, HALF], BF16)

    EXP = mybir.ActivationFunctionType.Exp
    MUL = mybir.AluOpType.mult
    ADD = mybir.AluOpType.add

    with tile.TileContext(nc) as tc:
        with (
            tc.tile_pool(name="const", bufs=1) as cpool,
            tc.tile_pool(name="vpool", bufs=1) as vpool,
            tc.tile_pool(name="otpool", bufs=1) as otpool,
            tc.tile_pool(name="sg", bufs=1) as sgpool,
            tc.tile_pool(name="bc", bufs=1) as bcpool,
        ):
            # ------------- PE warm-up first: DMA-free via memset ----------
            wm_t = cpool.tile([128, 128], BF16, tag="wm")
            nc.vector.memset(wm_t[:], 1.0)

            # ---------------- loads ----------
            xTb_t = cpool.tile([128, 4 * NJROWS], BF16, tag="xTb")
            xv = xTb_t[:].rearrange("p (dt j) -> p dt j", dt=4)
            xsv = xTs.ap().rearrange("p (dt j) -> p dt j", dt=4)
            for blk in range(NBLK):
                nc.sync.dma_start(
                    xv[:, :, PAD + blk * 128:PAD + (blk + 1) * 128],
                    xsv[:, :, blk * 128:(blk + 1) * 128],
                )
            Wvb_t = cpool.tile([128, 4 * D], BF16, tag="Wvb")
            nc.scalar.dma_start(Wvb_t[:], Wvb[:, :])
            Wsb_t = cpool.tile([128, 4 * H], BF16, tag="Wsb")
            nc.gpsimd.dma_start(Wsb_t[:], Wsb[:, :])
            m2r_t = cpool.tile([128, 256], BF16, tag="m2r")
            nc.gpsimd.dma_start(m2r_t[:], m2r[:, :])
            nc.sync.dma_start(
                xv[:, :, PAD - 64:PAD],
                xTp[:].rearrange("p (dt e j) -> p dt e j", dt=4, e=2)[:, :, 0, :],
            )
            nc.sync.dma_start(
                xv[:, :, PAD + HALF:PAD + HALF + 64],
                xTp[:].rearrange("p (dt e j) -> p dt e j", dt=4, e=2)[:, :, 1, :],
            )
            ones_t = cpool.tile([1, 128], BF16, tag="ones")
            nc.gpsimd.dma_start(ones_t[:], ones1[:, :])
            expnb_t = cpool.tile([128, NBLK * H], F32, tag="expnb")
            nc.gpsimd.dma_start(expnb_t[:], expnb[:, :])
            identb_t = cpool.tile([128, 128], BF16, tag="identb")
            nc.gpsimd.dma_start(identb_t[:], identb[:, :])
            ivp1_t = cpool.tile([128, NBLK * H], F32, tag="ivp1")
            nc.gpsimd.dma_start(ivp1_t[:], ivp1[:, :])
            ivnm_t = cpool.tile([128, NBLK * H], F32, tag="ivnm")
            nc.gpsimd.dma_start(ivnm_t[:], ivnm[:, :])
            selb_t = cpool.tile([16, 16 * 128], BF16, tag="selb")
            nc.gpsimd.dma_start(selb_t[:], selb[:, :])
            Wob_t = cpool.tile([128, 4 * D], BF16, tag="Wob")
            boutr_t = cpool.tile([1, D], BF16, tag="boutr")

            def xT(dt, c0, c1):
                return xTb_t[:, dt * NJROWS + c0:dt * NJROWS + c1]

            with tc.tile_pool(name="wm", bufs=1, space="PSUM") as wmp:
                wps = wmp.tile([128, 128], F32, tag="wps")
                for _ in range(16):
                    nc.tensor.matmul(
                        wps[:], lhsT=wm_t[:], rhs=wm_t[:],
                        start=True, stop=True,
                    )

            # ------------- sigma matmuls + r chain ------------------------
            with tc.tile_pool(name="pss", bufs=1, space="PSUM") as pss:
                sps = pss.tile([128, NBLK * H], F32, tag="sps")
                for b in range(NBLK):
                    for dt in range(4):
                        nc.tensor.matmul(
                            sps[:, b * H:(b + 1) * H],
                            lhsT=xT(dt, PAD + b * 128, PAD + (b + 1) * 128),
                            rhs=Wsb_t[:, dt * H:(dt + 1) * H],
                            start=(dt == 0), stop=(dt == 3),
                        )

                # sigma = 1/(1+exp(-s)*exp(-b)); r = 1/(1+exp(sigma))
                e1 = sgpool.tile([128, NBLK * H], F32, tag="e1")
                nc.scalar.activation(e1[:], sps[:], EXP, scale=-1.0)
                t0 = sgpool.tile([128, NBLK * H], F32, tag="t0")
                nc.vector.tensor_mul(t0[:], e1[:], expnb_t[:])
                nc.vector.tensor_scalar(t0[:], t0[:], 1.0, None, ADD)
                sig = sgpool.tile([128, NBLK * H], F32, tag="sig")
                nc.vector.reciprocal(sig[:], t0[:])
                e2 = sgpool.tile([128, NBLK * H], F32, tag="e2")
                nc.scalar.activation(e2[:], sig[:], EXP)
                t1 = sgpool.tile([128, NBLK * H], F32, tag="t1")
                nc.vector.tensor_scalar(t1[:], e2[:], 1.0, None, ADD)
                r_all = sgpool.tile([128, NBLK * H], F32, tag="r_all")
                nc.vector.reciprocal(r_all[:], t1[:])
                r_hb = sgpool.tile([128, NBLK * H], BF16, tag="r_hb")
                nc.vector.tensor_copy(
                    r_hb[:].rearrange("p (h b) -> p h b", b=NBLK),
                    r_all[:].rearrange("p (b h) -> p h b", h=H),
                )

            vtx = tc.tile_pool(name="psv", bufs=2, space="PSUM")
            psv = vtx.__enter__()
            V_all = vpool.tile([128, NVT * D], BF16, tag="V_all")
            vph = {}

            def v_mm(vt_i):
                pv = psv.tile([128, D], F32, tag="pv", name=f"pv{vt_i}")
                vph[vt_i] = pv
                k = 2 * vt_i + 1
                for dt in range(4):
                    nc.tensor.matmul(
                        pv[:],
                        lhsT=xT(dt, 64 * k, 64 * k + 128),
                        rhs=Wvb_t[:, dt * D:(dt + 1) * D],
                        start=(dt == 0), stop=(dt == 3),
                    )

            def v_copy(vt_i, eng="v"):
                dst = V_all[:, vt_i * D:(vt_i + 1) * D]
                src_ = vph.pop(vt_i)[:]
                if eng == "v":
                    nc.vector.tensor_copy(dst, src_)
                else:
                    nc.scalar.copy(dst, src_)

            v_mm(1)
            v_mm(2)

            # ---- stage r -> DRAM; broadcast helpers ----------------------
            R_all = bcpool.tile([128, H * HALF], BF16, tag="R_all")
            Iv_pair = bcpool.tile([128, 4 * HALF], BF16, tag="Iv_pair")

            def r_bcast(ch, h, eng):
                eng.dma_start(
                    R_all[:, h * HALF + ch * 512:h * HALF + (ch + 1) * 512],
                    r_d[h:h + 1, ch * 512:(ch + 1) * 512].to_broadcast((128, 512)),
                )

            def iv_bcast(hp, ch, eng):
                c0, c1 = ch * 512, (ch + 1) * 512
                eng.dma_start(
                    Iv_pair[0:64, hp * HALF + c0:hp * HALF + c1],
                    inv_d[2 * hp:2 * hp + 1, c0:c1].to_broadcast((64, 512)),
                )
                eng.dma_start(
                    Iv_pair[64:128, hp * HALF + c0:hp * HALF + c1],
                    inv_d[2 * hp + 1:2 * hp + 2, c0:c1].to_broadcast((64, 512)),
                )

            with tc.tile_pool(name="pst", bufs=1, space="PSUM") as pst:
                ptn = pst.tile([64, 128], BF16, tag="ptn")
                nc.tensor.transpose(ptn[:], r_hb[:], identb_t[:])
                rT = sgpool.tile([64, 128], BF16, tag="rT")
                nc.vector.tensor_copy(rT[:], ptn[:])
                nc.sync.dma_start(
                    r_d.ap().rearrange("h (b p) -> (h b) p", p=128), rT[:, :]
                )
                r_bcast(0, 0, nc.sync)
                r_bcast(0, 1, nc.gpsimd)

                # first head pair of chunk 0: partition-broadcast r via K=1
                # ones outer-product straight into PSUM -- no DRAM roundtrip,
                # so ARG00 is ready ~10us before the DMA broadcasts land.
                R0p = pst.tile([128, 1024], F32, tag="R0p")
                for hh in range(2):
                    for bb in range(4):
                        k = hh * 8 + bb
                        nc.tensor.matmul(
                            R0p[:, hh * 512 + bb * 128:hh * 512 + (bb + 1) * 128],
                            lhsT=selb_t[:, k * 128:(k + 1) * 128],
                            rhs=rT[0:16, :],
                            start=True, stop=True,
                        )

                v_mm(3)
                v_mm(4)

                # ---- peeled ARG/Exp (ch0, hp0): ahead of inv chain on DVE
                ARG00 = sgpool.tile([128, 2048], BF16, tag="ARG00")
                for hh in range(2):
                    R0 = R0p[:, hh * 512:(hh + 1) * 512]
                    nc.vector.tensor_tensor(
                        ARG00[:, hh * 1024:(hh + 1) * 1024]
                        .rearrange("p (b o q) -> p b o q", b=4, o=2),
                        m2r_t[:]
                        .rearrange("p (one o q) -> p one o q", one=1, o=2)
                        .broadcast_to((128, 4, 2, 128)),
                        R0.rearrange("p (b one q) -> p b one q", b=4, one=1)
                        .broadcast_to((128, 4, 2, 128)),
                        op=MUL,
                    )
                    v_copy(1 + hh)
                Q00 = sgpool.tile([128, 2048], BF16, tag="Q00")
                nc.scalar.activation(Q00[:], ARG00[:], EXP)

                # ---- 1/den chain: u = 1 + z - expA - expB; inv = (1-z)/u --
                z = sgpool.tile([128, NBLK * H], F32, tag="z")
                nc.scalar.activation(z[:], r_all[:], EXP, scale=-1.0)
                argA = sgpool.tile([128, NBLK * H], F32, tag="argA")
                nc.vector.tensor_mul(argA[:], r_all[:], ivp1_t[:])
                expA = sgpool.tile([128, NBLK * H], F32, tag="expA")
                nc.scalar.activation(expA[:], argA[:], EXP)
                argB = sgpool.tile([128, NBLK * H], F32, tag="argB")
                nc.vector.tensor_mul(argB[:], r_all[:], ivnm_t[:])
                expB = sgpool.tile([128, NBLK * H], F32, tag="expB")
                nc.scalar.activation(expB[:], argB[:], EXP)
                nc.scalar.dma_start(Wob_t[:], Wob[:, :])
                nc.scalar.dma_start(boutr_t[:], boutr[:, :])
                t2 = sgpool.tile([128, NBLK * H], F32, tag="t2")
                nc.vector.tensor_sub(t2[:], z[:], expA[:])
                nc.vector.tensor_sub(t2[:], t2[:], expB[:])
                u = sgpool.tile([128, NBLK * H], F32, tag="u")
                nc.vector.tensor_scalar(u[:], t2[:], 1.0, None, ADD)
                ru = sgpool.tile([128, NBLK * H], F32, tag="ru")
                nc.vector.reciprocal(ru[:], u[:])
                w = sgpool.tile([128, NBLK * H], F32, tag="w")
                nc.vector.tensor_scalar(w[:], z[:], -1.0, 1.0, MUL, ADD)
                inv_hb = sgpool.tile([128, NBLK * H], BF16, tag="inv_hb")
                nc.vector.tensor_tensor(
                    inv_hb[:].rearrange("p (h b) -> p h b", b=NBLK),
                    w[:].rearrange("p (b h) -> p h b", h=H),
                    ru[:].rearrange("p (b h) -> p h b", h=H),
                    op=MUL,
                )
                pti = pst.tile([64, 128], BF16, tag="pti")
                nc.tensor.transpose(pti[:], inv_hb[:], identb_t[:])
                iT = sgpool.tile([64, 128], BF16, tag="iT")
                nc.vector.tensor_copy(iT[:], pti[:])
                nc.gpsimd.dma_start(
                    inv_d.ap().rearrange("h (b p) -> (h b) p", p=128), iT[:, :]
                )

            iv_bcast(0, 0, nc.gpsimd)
            r_bcast(0, 2, nc.sync)
            r_bcast(0, 3, nc.gpsimd)

            v_mm(0)
            v_copy(3)
            v_copy(4)
            v_copy(0, "s")

            outT_t = []
            for t in range(4):
                oT = otpool.tile([128, HALF], BF16, tag=f"oT{t}")
                outT_t.append(oT)

            def Vs(t, h):
                return V_all[:, t * D + h * 64:t * D + (h + 1) * 64]

            # ---------------- main loop (2 chunks of 4 i-blocks) ----------
            with (
                tc.tile_pool(name="ap", bufs=3) as apool,
                tc.tile_pool(name="qp", bufs=3) as qpool,
                tc.tile_pool(name="fin", bufs=3) as fpool,
                tc.tile_pool(name="psa", bufs=1, space="PSUM") as psa,
                tc.tile_pool(name="psf", bufs=4, space="PSUM") as psf,
            ):
                pfs = {}

                def arg_exp(ch, hp):
                    ARG = apool.tile(
                        [128, 2048], BF16, tag="ARG", name=f"A{ch}{hp}"
                    )
                    for hh in range(2):
                        h = 2 * hp + hh
                        R = R_all[:, h * HALF + ch * 512:
                                  h * HALF + (ch + 1) * 512]
                        nc.vector.tensor_tensor(
                            ARG[:, hh * 1024:(hh + 1) * 1024]
                            .rearrange("p (b o q) -> p b o q", b=4, o=2),
                            m2r_t[:]
                            .rearrange("p (one o q) -> p one o q", one=1, o=2)
                            .broadcast_to((128, 4, 2, 128)),
                            R.rearrange("p (b one q) -> p b one q", b=4, one=1)
                            .broadcast_to((128, 4, 2, 128)),
                            op=MUL,
                        )
                    Q = qpool.tile([128, 2048], BF16, tag="Q", name=f"Q{ch}{hp}")
                    nc.scalar.activation(Q[:], ARG[:], EXP)
                    return Q

                def av_norm(ch, hp, Q):
                    pav = psa.tile(
                        [128, 512], F32, tag="pav", name=f"pav{ch}{hp}"
                    )
                    for bi in range(4):
                        b = ch * 4 + bi
                        c0 = bi * 128
                        for hh in range(2):
                            h = 2 * hp + hh
                            for o in range(2):
                                nc.tensor.matmul(
                                    pav[hh * 64:(hh + 1) * 64, c0:c0 + 128],
                                    lhsT=Vs(b + o, h),
                                    rhs=Q[:, hh * 1024 + bi * 256 + o * 128:
                                          hh * 1024 + bi * 256 + (o + 1) * 128],
                                    start=(o == 0),
                                    stop=(o == 1),
                                )
                    nc.vector.tensor_tensor(
                        outT_t[hp][:, ch * 512:(ch + 1) * 512],
                        pav[:],
                        Iv_pair[:, hp * HALF + ch * 512:
                                hp * HALF + (ch + 1) * 512],
                        op=MUL,
                    )

                def proj_acc(ch, hp):
                    # accumulate this head pair into each block's projection
                    for bi in range(4):
                        b = ch * 4 + bi
                        cols = slice(b * 128, (b + 1) * 128)
                        if hp == 0:
                            pfs[b] = psf.tile(
                                [128, D], F32, tag="pf", name=f"pf{b}"
                            )
                        nc.tensor.matmul(
                            pfs[b][:],
                            lhsT=outT_t[hp][:, cols],
                            rhs=Wob_t[:, hp * D:(hp + 1) * D],
                            start=(hp == 0),
                            stop=False,
                        )
                        if hp == 3:
                            nc.tensor.matmul(
                                pfs[b][:], lhsT=ones_t[:], rhs=boutr_t[:],
                                start=False, stop=True,
                            )

                def do_fin(b):
                    cols = slice(b * 128, (b + 1) * 128)
                    fin = fpool.tile([128, D], BF16, tag="fin", name=f"fin{b}")
                    nc.vector.tensor_copy(fin[:], pfs.pop(b)[:])
                    nc.sync.dma_start(out[cols, :], fin[:])

                av_norm(0, 0, Q00)
                proj_acc(0, 0)

                for hp in range(1, 4):
                    nh = 2 * hp + 2
                    if nh < H:
                        r_bcast(0, nh, nc.sync)
                        r_bcast(0, nh + 1, nc.gpsimd)
                    else:
                        r_bcast(1, 0, nc.sync)
                        r_bcast(1, 1, nc.gpsimd)
                    iv_bcast(hp, 0, nc.gpsimd)
                    Q = arg_exp(0, hp)
                    v_mm(4 + hp)
                    av_norm(0, hp, Q)
                    v_copy(4 + hp)
                    proj_acc(0, hp)
                v_mm(8)
                v_copy(8)

                for hp in range(4):
                    nh = 2 * hp + 2
                    if nh < H:
                        r_bcast(1, nh, nc.sync)
                        r_bcast(1, nh + 1, nc.gpsimd)
                    iv_bcast(hp, 1, nc.gpsimd)
                    Q = arg_exp(1, hp)
                    do_fin(hp)  # chunk-0 fin: after the ARG so Exps stay fed
                    av_norm(1, hp, Q)
                    proj_acc(1, hp)
                for b in range(4, 8):
                    do_fin(b)

            vtx.__exit__(None, None, None)

    nc.compile()
    return nc


def _make_in_maps(x, W_v, W_sigma, b_sigma, W_out, b_out):
    bf = ml_dtypes.bfloat16
    m2r1 = np.empty((128, 256), dtype=np.float32)
    p = np.arange(128, dtype=np.float32)[:, None]
    q = np.arange(128, dtype=np.float32)[None, :]
    for o in range(2):
        m2r1[:, o * 128:(o + 1) * 128] = -np.abs(q - p + 64.0 - 128.0 * o)
    m2r = m2r1.astype(bf)
    identb = np.eye(128, dtype=np.float32).astype(bf)

    def dt_interleave(W):  # [512, F] -> [128, 4*F]
        F = W.shape[1]
        return np.ascontiguousarray(
            W.reshape(4, 128, F).transpose(1, 0, 2).reshape(128, 4 * F)
        )

    Wvb = dt_interleave(W_v.astype(bf))
    Wsb = dt_interleave(W_sigma.astype(bf))
    Wob = dt_interleave(W_out.astype(bf))
    ones1 = np.ones((1, 128), dtype=np.float32).astype(bf)
    selb = np.zeros((16, 16 * 128), dtype=np.float32)
    for k in range(16):
        selb[k, k * 128:(k + 1) * 128] = 1.0
    selb = selb.astype(bf)
    # exp(-b_sigma) replicated in (block, head) column layout
    expnb = np.tile(np.exp(-b_sigma)[None, :], (128, NBLK)).astype(np.float32)
    boutr = b_out[None, :].astype(bf)

    in_maps = []
    for c in range(8):
        bb, half = c // 2, c % 2
        i_start = half * HALF
        xp = np.zeros((NJROWS, D), dtype=np.float32)
        j_lo = max(0, i_start - PAD)
        j_hi = min(N, i_start + HALF + PAD)
        xp[j_lo - (i_start - PAD):j_hi - (i_start - PAD)] = x[bb, j_lo:j_hi]
        xpT = np.ascontiguousarray(xp.T).astype(bf)           # [512, NJROWS]
        xTs = dt_interleave(xpT[:, PAD:PAD + HALF])           # [128, 4*1024]
        xTp = dt_interleave(np.concatenate(
            [xpT[:, PAD - 64:PAD], xpT[:, PAD + HALF:PAD + HALF + 64]], axis=1
        ))                                                     # [128, 4*128]

        pcol = np.arange(128, dtype=np.float32)[:, None]
        blk = np.arange(NBLK, dtype=np.float32)[None, :]
        i_abs = i_start + blk * 128 + pcol                     # [128, NBLK]
        ivp1 = np.repeat(-(i_abs + 1.0), H, axis=1).astype(np.float32)
        ivnm = np.repeat(-(float(N) - i_abs), H, axis=1).astype(np.float32)

        in_maps.append(
            {
                "xTs": xTs,
                "xTp": xTp,
                "Wvb": Wvb,
                "Wsb": Wsb,
                "Wob": Wob,
                "ones1": ones1,
                "expnb": expnb,
                "boutr": boutr,
                "m2r": m2r,
                "ivp1": ivp1,
                "ivnm": ivnm,
                "identb": identb,
                "selb": selb,
            }
        )
    return in_maps


def kernel(x, W_v, W_sigma, b_sigma, W_out, b_out):
    global _nc_cache
    x = np.asarray(x, dtype=np.float32)
    W_v = np.asarray(W_v, dtype=np.float32)
    W_sigma = np.asarray(W_sigma, dtype=np.float32)
    b_sigma = np.asarray(b_sigma, dtype=np.float32)
    W_out = np.asarray(W_out, dtype=np.float32)
    b_out = np.asarray(b_out, dtype=np.float32)

    if _nc_cache is None:
        _nc_cache = _build_nc()
    nc = _nc_cache

    in_maps = _make_in_maps(x, W_v, W_sigma, b_sigma, W_out, b_out)
    res = run_bass_kernel_spmd(nc, in_maps, core_ids=list(range(8)))

    out = np.empty((B, N, D), dtype=np.float32)
    for c in range(8):
        bb, half = c // 2, c % 2
        out[bb, half * HALF:(half + 1) * HALF, :] = (
            res.results[c]["out"].astype(np.float32)
        )
    return out
